# revision 17
# baseline (speedup 1.0000x reference)
"""Trainium2 Bass kernel for nn_KGLearner (gnn_message_passing).

Math (per reference):
    s_proj = subevent @ attn_w[:D]          # [S]
    e_proj = event @ attn_w[D:]             # [E]
    scores = leaky_relu(adj * (e_proj[:,None] + s_proj[None,:]), 0.2)
    attn   = softmax(scores, -1)
    out    = (event + (attn*adj) @ subevent) * 0.5

Key identities used on device:
    leaky(adj*u) = adj*leaky(u)       (adj >= 0)
    softmax without max-subtraction   (scores bounded, |t|<10, exp safe in fp16)

Sharding: row-wise over num_evt, 8 cores x 1024 rows. subevent replicated.

Device pipeline per core, [s, ev] layout (s on partitions after a PE
transpose of adj tiles), iterating over 128 column-slices of 128 s each:
    DMA   adj[:, s-slice] (fp32)                            -> SBUF
    GpSimd cast fp32->fp16                                  -> SBUF
    PE    8x transpose 128x128 (fp16)                       -> PSUM  adjT
    DVE   u  = (ebt + s_proj[sc])            (tensor_scalar, per-chunk)
    DVE   L  = max(0.2*u, u)                 (scalar_tensor_tensor)
    DVE   t  = adjT * L                      (tensor_tensor, PSUM src)
    ACT   p  = Exp(t)
    DVE   w  = adjT * p                      (tensor_tensor, PSUM src)
    PE    pv[b]  += w[:,b-block].T @ sub16[s-chunk]          (PSUM accum)
    PE    rs[b]  += p[:,b-block].T @ ones                    (PSUM accum)
Epilogue: out = pv * (0.5/rs) + 0.5*event   (reciprocal + STT), DMA out.

Projections s_proj/e_proj (0.01% of FLOPs) are computed on host.

Wait-slot notes: walrus allows ONE sync wait per instruction (2 on
InstEventSemaphore). Bacc.compile()'s generate_event_semaphores splits
excess waits; tiny Copy-activation "touch" ops keep the hot DMAs at a
single wait so no evsem chain lands on the DMA critical path.
"""

import os
import numpy as np

CAST_ENGINE = os.environ.get("KGL_CAST", "gpsimd")

E_TOT = 8192
S_TOT = 16384
D = 128
N_CORES = 8
EV = E_TOT // N_CORES          # 1024 event rows per core
EVB = EV // 128                # 8 ev blocks of 128
SC_TOT = S_TOT // 128          # 128 s-chunks of 128
S_PER_IT = 128                 # one s-chunk per loop iteration
N_IT = S_TOT // S_PER_IT       # 128

# dtype for the streamed pipeline (fp16: 11-bit mantissa, exp(t)<e^10<<65504)
_DT_NP = np.float16

_CACHE = {}


def _build_nc(repeat=1):
    import concourse.bass as bass
    import concourse.bacc as bacc
    import concourse.mybir as mybir
    import concourse.tile as tile
    from concourse.tile_rust import add_dep_helper
    from concourse.masks import make_identity
    from contextlib import ExitStack, nullcontext

    f32 = mybir.dt.float32
    f16 = mybir.dt.float16
    Alu = mybir.AluOpType
    Act = mybir.ActivationFunctionType

    nc = bacc.Bacc()

    adj_in = nc.declare_dram_parameter("adj", [EV, S_TOT], f32, isOutput=False)
    sub_in = nc.declare_dram_parameter("subt", [128, SC_TOT * D], f16, isOutput=False)
    spj_in = nc.declare_dram_parameter("spj", [128, SC_TOT], f32, isOutput=False)
    ebt_in = nc.declare_dram_parameter("ebt", [128, EV], f16, isOutput=False)
    evh_in = nc.declare_dram_parameter("evh", [128, EVB * D], f32, isOutput=False)
    out_t = nc.declare_dram_parameter("out", [128, EVB * D], f32, isOutput=True)

    with ExitStack() as ctx:
        tc = ctx.enter_context(tile.TileContext(nc))
        singles = ctx.enter_context(tc.tile_pool(name="singles", bufs=1))
        stagea = ctx.enter_context(tc.tile_pool(name="stagea", bufs=8))
        stageb = ctx.enter_context(tc.tile_pool(name="stageb", bufs=4))
        mid = ctx.enter_context(tc.tile_pool(name="mid", bufs=4))
        ppool = ctx.enter_context(tc.tile_pool(name="ppsum", bufs=3, space="PSUM"))
        accum = ctx.enter_context(tc.tile_pool(name="accum", bufs=1, space="PSUM"))
        outp = ctx.enter_context(tc.tile_pool(name="outp", bufs=1))

        # ---- prologue: constants ----
        # All prologue producers go through gpsimd so iteration-0 consumers
        # need only one cross-engine wait each.
        sub_sb = singles.tile([128, SC_TOT * D], f16)
        nc.gpsimd.dma_start(out=sub_sb, in_=sub_in[:, :])
        spj_sb = singles.tile([128, SC_TOT], f32)
        nc.gpsimd.dma_start(out=spj_sb, in_=spj_in[:, :])
        ebt_sb = singles.tile([128, EV], f16)
        nc.gpsimd.dma_start(out=ebt_sb, in_=ebt_in[:, :])
        evh_sb = singles.tile([128, EVB * D], f32)
        nc.gpsimd.dma_start(out=evh_sb, in_=evh_in[:, :])

        ident = singles.tile([128, 128], f16)
        make_identity(nc, ident)
        ones_col = singles.tile([128, 1], f16)
        nc.gpsimd.memset(ones_col, 1.0)

        # "Touch" every prologue tile from DVE: later consumers (incl. the
        # wait-slot-poor TensorScalarPtr ops and PE matmuls that already wait
        # on DVE) then never need a fresh DMA-queue wait.
        junk = singles.tile([128, 4], f32)
        nc.vector.tensor_copy(junk[:, 0:1], spj_sb[:, 0:1])
        nc.vector.tensor_copy(junk[:, 1:2], ebt_sb[:, 0:1])
        nc.vector.tensor_copy(junk[:, 2:3], evh_sb[:, 0:1])
        nc.vector.tensor_copy(junk[:, 3:4], sub_sb[:, 0:1])

        pv_ps = accum.tile([128, EVB * D], f32)   # 4KB/part = 2 banks
        rs_ps = accum.tile([128, EVB], f32)

        adj_r = adj_in.rearrange("(b p) s -> p b s", p=128)  # [128, EVB, S]
        junk2 = singles.tile([128, 1], f32)

        state = {"last_pv": None}

        def emit_iter(it, casts, rs_mms):
            sc = it
            s0 = it * S_PER_IT
            # DMACopy has one sync-wait slot, so the adj loads are issued from
            # the Activation sequencer, where a tiny Copy-activation "touch"
            # (same table as Exp) first absorbs the Pool (cast slot-release)
            # and PE (pt slot-release) waits; the DMA then only carries its
            # HW-queue WAW wait.
            touch = nc.scalar.activation(junk2, junk[:, 0:1], Act.Copy)
            if it >= 8:
                add_dep_helper(touch.ins, casts[it - 8].ins, sync=True,
                               reason="absorb pool slot wait")
            if it >= 4:
                add_dep_helper(touch.ins, rs_mms[it - 4].ins, sync=True,
                               reason="absorb PE pt-slot wait")
            adj_sb = stagea.tile([128, EVB, S_PER_IT], f32, tag="adjf32")
            dma_i = nc.scalar.dma_start(
                out=adj_sb, in_=adj_r[:, :, s0:s0 + S_PER_IT])
            add_dep_helper(dma_i.ins, touch.ins, sync=False,
                           reason="order touch before dma")

            adj16 = stageb.tile([128, EVB, S_PER_IT], f16, tag="adjf16")
            if CAST_ENGINE == "gpsimd":
                casts.append(nc.gpsimd.tensor_copy(adj16, adj_sb))
            elif CAST_ENGINE == "vector":
                casts.append(nc.vector.tensor_copy(adj16, adj_sb))
            elif CAST_ENGINE.startswith("split"):
                k = int(CAST_ENGINE[5:])
                if it % 8 < k:
                    casts.append(nc.scalar.activation(adj16, adj_sb, Act.Copy))
                else:
                    casts.append(nc.vector.tensor_copy(adj16, adj_sb))
            else:
                casts.append(nc.scalar.activation(adj16, adj_sb, Act.Copy))

            # PE transpose into PSUM: adjT[p, b*128+e] = adj[b*128+e, s0+p]
            adjT = ppool.tile([128, EV], f16, tag="adjT")
            for b in range(EVB):
                nc.tensor.transpose(
                    adjT[:, b * 128:(b + 1) * 128], adj16[:, b, :], ident)

            # u = ebt + s_proj[sc]  (per-partition scalar bias)
            u = mid.tile([128, EV], f16, tag="u")
            nc.vector.tensor_scalar(
                u, ebt_sb, spj_sb[:, sc:sc + 1], None, Alu.add)
            # L = max(0.2u, u) = leaky_relu(u, 0.2)
            lk = mid.tile([128, EV], f16, tag="lk")
            nc.vector.scalar_tensor_tensor(lk, u, 0.2, u, Alu.mult, Alu.max)
            # t = adjT * L
            t = mid.tile([128, EV], f16, tag="t")
            nc.vector.tensor_tensor(t, adjT, lk, Alu.mult)
            # p = exp(t)
            pt = mid.tile([128, EV], f16, tag="pt")
            nc.scalar.activation(pt, t, Act.Exp)
            # w = adjT * p
            w = mid.tile([128, EV], f16, tag="w")
            nc.vector.tensor_tensor(w, adjT, pt, Alu.mult)

            first = sc == 0
            last = sc == SC_TOT - 1
            for b in range(EVB):
                mm = nc.tensor.matmul(
                    pv_ps[:, b * D:(b + 1) * D],
                    lhsT=w[:, b * 128:(b + 1) * 128],
                    rhs=sub_sb[:, sc * D:(sc + 1) * D],
                    start=first, stop=last)
                if last:
                    state["last_pv"] = mm
                rs_mm = nc.tensor.matmul(
                    rs_ps[:, b:b + 1],
                    lhsT=pt[:, b * 128:(b + 1) * 128],
                    rhs=ones_col,
                    start=first, stop=last)
            rs_mms.append(rs_mm)

        rep_ctx = tc.For_i(0, repeat, 1) if repeat > 1 else nullcontext()
        with rep_ctx:
            casts = []
            rs_mms = []
            for it in range(N_IT):
                emit_iter(it, casts, rs_mms)

        # ---- epilogue ----
        rinv = outp.tile([128, EVB], f32)
        recip_i = nc.vector.reciprocal(rinv, rs_ps)
        add_dep_helper(recip_i.ins, state["last_pv"].ins, sync=True,
                       reason="cover pv stop before epilogue STT")
        rinv05 = outp.tile([128, EVB], f32)
        nc.vector.tensor_scalar(rinv05, rinv, 0.5, None, Alu.mult)
        out_sb = outp.tile([128, EVB * D], f32)
        last_stt = None
        for b in range(EVB):
            last_stt = nc.vector.scalar_tensor_tensor(
                out_sb[:, b * D:(b + 1) * D],
                pv_ps[:, b * D:(b + 1) * D],
                rinv05[:, b:b + 1],
                evh_sb[:, b * D:(b + 1) * D],
                Alu.mult, Alu.add)
        touch_out = nc.scalar.activation(junk2, junk[:, 0:1], Act.Copy)
        add_dep_helper(touch_out.ins, last_stt.ins, sync=True,
                       reason="absorb DVE wait for out dma")
        dma_o = nc.scalar.dma_start(out=out_t[:, :], in_=out_sb)
        add_dep_helper(dma_o.ins, touch_out.ins, sync=False,
                       reason="order touch before out dma")

    # Full bacc lowering: splits multi-wait sync_info into EventSemaphore
    # chains (HW allows one wait per instruction), allocates registers, etc.
    nc.compile()
    return nc


def _get_nc(repeat=1):
    key = ("nc", repeat)
    if key not in _CACHE:
        _CACHE[key] = _build_nc(repeat)
    return _CACHE[key]


def _prep(adj, subevent, event, attn_w):
    adj = np.ascontiguousarray(adj, dtype=np.float32)
    subevent = np.ascontiguousarray(subevent, dtype=np.float32)
    event = np.ascontiguousarray(event, dtype=np.float32)
    attn_w = np.asarray(attn_w, dtype=np.float32)

    a_s, a_e = attn_w[:D], attn_w[D:]
    s_proj = (subevent @ a_s).astype(np.float32)        # [S]
    e_proj = (event @ a_e).astype(np.float32)           # [E]

    # sub16[p, n*D+d] = subevent[n*128+p, d]
    sub16 = (
        subevent.astype(_DT_NP)
        .reshape(SC_TOT, 128, D).transpose(1, 0, 2).reshape(128, SC_TOT * D)
    )
    sub16 = np.ascontiguousarray(sub16)
    # spj[p, n] = s_proj[n*128+p]
    spj = np.ascontiguousarray(s_proj.reshape(SC_TOT, 128).T)

    in_maps = []
    for c in range(N_CORES):
        sl = slice(c * EV, (c + 1) * EV)
        ebt = np.ascontiguousarray(
            np.broadcast_to(e_proj[sl].astype(_DT_NP)[None, :], (128, EV)))
        evh = np.ascontiguousarray(
            (0.5 * event[sl])
            .astype(np.float32)
            .reshape(EVB, 128, D).transpose(1, 0, 2).reshape(128, EVB * D))
        in_maps.append({
            "adj": adj[sl],
            "subt": sub16,
            "spj": spj,
            "ebt": ebt,
            "evh": evh,
        })
    return in_maps


def _make_in_maps(inputs):
    return _prep(inputs["adj"], inputs["subevent"], inputs["event"],
                 inputs["attn_w"])


def kernel(adj, subevent, event, attn_w):
    from concourse.bass_utils import run_bass_kernel_spmd

    in_maps = _prep(adj, subevent, event, attn_w)
    nc = _get_nc()
    res = run_bass_kernel_spmd(nc, in_maps, list(range(N_CORES)))

    out = np.empty((E_TOT, D), dtype=np.float32)
    for c in range(N_CORES):
        o = res.results[c]["out"]  # [128, EVB*D]
        out[c * EV:(c + 1) * EV] = (
            o.reshape(128, EVB, D).transpose(1, 0, 2).reshape(EV, D)
        )
    return out


if __name__ == "__main__":
    rng = np.random.default_rng(0)
    adj = rng.random((E_TOT, S_TOT), dtype=np.float32)
    sub = rng.standard_normal((S_TOT, D), dtype=np.float32)
    ev = rng.standard_normal((E_TOT, D), dtype=np.float32)
    w = rng.uniform(-0.1, 0.1, 2 * D).astype(np.float32)
    out = kernel(adj, sub, ev, w)
    print(out.shape, out.dtype)


# revision 18
# speedup vs baseline: 4.8285x; 4.8285x over previous
"""Trainium2 Bass kernel for nn_KGLearner (gnn_message_passing).

Math (per reference):
    s_proj = subevent @ attn_w[:D]          # [S]
    e_proj = event @ attn_w[D:]             # [E]
    scores = leaky_relu(adj * (e_proj[:,None] + s_proj[None,:]), 0.2)
    attn   = softmax(scores, -1)
    out    = (event + (attn*adj) @ subevent) * 0.5

Key identities used on device:
    leaky(adj*u) = adj*leaky(u)       (adj >= 0)
    softmax without max-subtraction   (scores bounded, |t|<10, exp safe in fp16)

Sharding: row-wise over num_evt, 8 cores x 1024 rows. subevent replicated.

Device pipeline per core, [s, ev] layout (s on partitions after a PE
transpose of adj tiles), iterating over 128 column-slices of 128 s each:
    DMA   adj[:, s-slice] (fp32)                            -> SBUF
    GpSimd cast fp32->fp16                                  -> SBUF
    PE    8x transpose 128x128 (fp16)                       -> PSUM  adjT
    DVE   u  = (ebt + s_proj[sc])            (tensor_scalar, per-chunk)
    DVE   L  = max(0.2*u, u)                 (scalar_tensor_tensor)
    DVE   t  = adjT * L                      (tensor_tensor, PSUM src)
    ACT   p  = Exp(t)
    DVE   w  = adjT * p                      (tensor_tensor, PSUM src)
    PE    pv[b]  += w[:,b-block].T @ sub16[s-chunk]          (PSUM accum)
    PE    rs[b]  += p[:,b-block].T @ ones                    (PSUM accum)
Epilogue: out = pv * (0.5/rs) + 0.5*event   (reciprocal + STT), DMA out.

Projections s_proj/e_proj (0.01% of FLOPs) are computed on host.

Wait-slot notes: walrus allows ONE sync wait per instruction (2 on
InstEventSemaphore). Bacc.compile()'s generate_event_semaphores splits
excess waits; tiny Copy-activation "touch" ops keep the hot DMAs at a
single wait so no evsem chain lands on the DMA critical path.
"""

import os
import numpy as np

CAST_ENGINE = os.environ.get("KGL_CAST", "gpsimd")

E_TOT = 8192
S_TOT = 16384
D = 128
N_CORES = 8
EV = E_TOT // N_CORES          # 1024 event rows per core
EVB = EV // 128                # 8 ev blocks of 128
SC_TOT = S_TOT // 128          # 128 s-chunks of 128
S_PER_IT = 128                 # one s-chunk per loop iteration
N_IT = S_TOT // S_PER_IT       # 128

# dtype for the streamed pipeline (fp16: 11-bit mantissa, exp(t)<e^10<<65504)
_DT_NP = np.float16

_CACHE = {}


def _build_nc(repeat=1):
    import concourse.bass as bass
    import concourse.bacc as bacc
    import concourse.mybir as mybir
    import concourse.tile as tile
    from concourse.tile_rust import add_dep_helper
    from concourse.masks import make_identity
    from contextlib import ExitStack, nullcontext

    f32 = mybir.dt.float32
    f16 = mybir.dt.float16
    Alu = mybir.AluOpType
    Act = mybir.ActivationFunctionType

    nc = bacc.Bacc()

    adj_in = nc.declare_dram_parameter("adj", [EV, S_TOT], f32, isOutput=False)
    sub_in = nc.declare_dram_parameter("subt", [128, SC_TOT * D], f16, isOutput=False)
    spj_in = nc.declare_dram_parameter("spj", [128, SC_TOT], f32, isOutput=False)
    ebt_in = nc.declare_dram_parameter("ebt", [128, EV], f16, isOutput=False)
    evh_in = nc.declare_dram_parameter("evh", [128, EVB * D], f32, isOutput=False)
    out_t = nc.declare_dram_parameter("out", [128, EVB * D], f32, isOutput=True)

    with ExitStack() as ctx:
        tc = ctx.enter_context(tile.TileContext(nc))
        singles = ctx.enter_context(tc.tile_pool(name="singles", bufs=1))
        stagea = ctx.enter_context(tc.tile_pool(name="stagea", bufs=8))
        stageb = ctx.enter_context(tc.tile_pool(name="stageb", bufs=4))
        mid = ctx.enter_context(tc.tile_pool(name="mid", bufs=3))
        ppool = ctx.enter_context(tc.tile_pool(name="ppsum", bufs=2, space="PSUM"))
        accum = ctx.enter_context(tc.tile_pool(name="accum", bufs=1, space="PSUM"))
        outp = ctx.enter_context(tc.tile_pool(name="outp", bufs=1))

        # ---- prologue: constants ----
        # All prologue producers go through gpsimd so iteration-0 consumers
        # need only one cross-engine wait each.
        sub_sb = singles.tile([128, SC_TOT * D], f16)
        nc.gpsimd.dma_start(out=sub_sb, in_=sub_in[:, :])
        spj_sb = singles.tile([128, SC_TOT], f32)
        nc.gpsimd.dma_start(out=spj_sb, in_=spj_in[:, :])
        ebt_sb = singles.tile([128, EV], f16)
        nc.gpsimd.dma_start(out=ebt_sb, in_=ebt_in[:, :])
        evh_sb = singles.tile([128, EVB * D], f32)
        nc.gpsimd.dma_start(out=evh_sb, in_=evh_in[:, :])

        ident = singles.tile([128, 128], f16)
        make_identity(nc, ident)
        ones_col = singles.tile([128, 1], f16)
        nc.gpsimd.memset(ones_col, 1.0)

        # "Touch" every prologue tile from DVE: later consumers (incl. the
        # wait-slot-poor TensorScalarPtr ops and PE matmuls that already wait
        # on DVE) then never need a fresh DMA-queue wait.
        junk = singles.tile([128, 4], f32)
        nc.vector.tensor_copy(junk[:, 0:1], spj_sb[:, 0:1])
        nc.vector.tensor_copy(junk[:, 1:2], ebt_sb[:, 0:1])
        nc.vector.tensor_copy(junk[:, 2:3], evh_sb[:, 0:1])
        nc.vector.tensor_copy(junk[:, 3:4], sub_sb[:, 0:1])

        pv_ps = accum.tile([128, EVB * D], f32)   # 4KB/part = 2 banks
        rs_ps = accum.tile([128, EVB], f32)

        adj_r = adj_in.rearrange("(b p) s -> p b s", p=128)  # [128, EVB, S]
        junk2 = singles.tile([128, 1], f32)

        state = {"last_pv": None}

        def emit_iter(it, casts, rs_mms):
            sc = it
            s0 = it * S_PER_IT
            # DMACopy has one sync-wait slot, so the adj loads are issued from
            # the Activation sequencer, where a tiny Copy-activation "touch"
            # (same table as Exp) first absorbs the Pool (cast slot-release)
            # and PE (pt slot-release) waits; the DMA then only carries its
            # HW-queue WAW wait.
            touch = nc.scalar.activation(junk2, junk[:, 0:1], Act.Copy)
            if it >= 8:
                add_dep_helper(touch.ins, casts[it - 8].ins, sync=True,
                               reason="absorb pool slot wait")
            if it >= 4:
                add_dep_helper(touch.ins, rs_mms[it - 4].ins, sync=True,
                               reason="absorb PE pt-slot wait")
            adj_sb = stagea.tile([128, EVB, S_PER_IT], f32, tag="adjf32")
            dma_i = nc.scalar.dma_start(
                out=adj_sb, in_=adj_r[:, :, s0:s0 + S_PER_IT])
            add_dep_helper(dma_i.ins, touch.ins, sync=False,
                           reason="order touch before dma")

            adj16 = stageb.tile([128, EVB, S_PER_IT], f16, tag="adjf16")
            if CAST_ENGINE == "gpsimd":
                casts.append(nc.gpsimd.tensor_copy(adj16, adj_sb))
            elif CAST_ENGINE == "vector":
                casts.append(nc.vector.tensor_copy(adj16, adj_sb))
            elif CAST_ENGINE.startswith("split"):
                k = int(CAST_ENGINE[5:])
                if it % 8 < k:
                    casts.append(nc.scalar.activation(adj16, adj_sb, Act.Copy))
                else:
                    casts.append(nc.vector.tensor_copy(adj16, adj_sb))
            else:
                casts.append(nc.scalar.activation(adj16, adj_sb, Act.Copy))

            # PE transpose into PSUM: adjT[p, b*128+e] = adj[b*128+e, s0+p]
            adjT = ppool.tile([128, EV], f16, tag="adjT")
            for b in range(EVB):
                nc.tensor.transpose(
                    adjT[:, b * 128:(b + 1) * 128], adj16[:, b, :], ident)

            # u = ebt + s_proj[sc]  (per-partition scalar bias)
            u = mid.tile([128, EV], f16, tag="u")
            nc.vector.tensor_scalar(
                u, ebt_sb, spj_sb[:, sc:sc + 1], None, Alu.add)
            # L = max(0.2u, u) = leaky_relu(u, 0.2)
            lk = mid.tile([128, EV], f16, tag="lk")
            nc.vector.scalar_tensor_tensor(lk, u, 0.2, u, Alu.mult, Alu.max)
            # t = adjT * L
            t = mid.tile([128, EV], f16, tag="t")
            nc.vector.tensor_tensor(t, adjT, lk, Alu.mult)
            # p = exp(t)
            pt = mid.tile([128, EV], f16, tag="pt")
            nc.scalar.activation(pt, t, Act.Exp)
            # w = adjT * p
            w = mid.tile([128, EV], f16, tag="w")
            nc.vector.tensor_tensor(w, adjT, pt, Alu.mult)

            first = sc == 0
            last = sc == SC_TOT - 1
            for b in range(EVB):
                mm = nc.tensor.matmul(
                    pv_ps[:, b * D:(b + 1) * D],
                    lhsT=w[:, b * 128:(b + 1) * 128],
                    rhs=sub_sb[:, sc * D:(sc + 1) * D],
                    start=first, stop=last)
                if last:
                    state["last_pv"] = mm
                rs_mm = nc.tensor.matmul(
                    rs_ps[:, b:b + 1],
                    lhsT=pt[:, b * 128:(b + 1) * 128],
                    rhs=ones_col,
                    start=first, stop=last)
            rs_mms.append(rs_mm)

        rep_ctx = tc.For_i(0, repeat, 1) if repeat > 1 else nullcontext()
        with rep_ctx:
            casts = []
            rs_mms = []
            for it in range(N_IT):
                emit_iter(it, casts, rs_mms)

        # ---- epilogue ----
        rinv = outp.tile([128, EVB], f32)
        recip_i = nc.vector.reciprocal(rinv, rs_ps)
        add_dep_helper(recip_i.ins, state["last_pv"].ins, sync=True,
                       reason="cover pv stop before epilogue STT")
        rinv05 = outp.tile([128, EVB], f32)
        nc.vector.tensor_scalar(rinv05, rinv, 0.5, None, Alu.mult)
        out_sb = outp.tile([128, EVB * D], f32)
        last_stt = None
        for b in range(EVB):
            last_stt = nc.vector.scalar_tensor_tensor(
                out_sb[:, b * D:(b + 1) * D],
                pv_ps[:, b * D:(b + 1) * D],
                rinv05[:, b:b + 1],
                evh_sb[:, b * D:(b + 1) * D],
                Alu.mult, Alu.add)
        touch_out = nc.scalar.activation(junk2, junk[:, 0:1], Act.Copy)
        add_dep_helper(touch_out.ins, last_stt.ins, sync=True,
                       reason="absorb DVE wait for out dma")
        dma_o = nc.scalar.dma_start(out=out_t[:, :], in_=out_sb)
        add_dep_helper(dma_o.ins, touch_out.ins, sync=False,
                       reason="order touch before out dma")

    # Full bacc lowering: splits multi-wait sync_info into EventSemaphore
    # chains (HW allows one wait per instruction), allocates registers, etc.
    nc.compile()
    return nc


def _get_nc(repeat=1):
    key = ("nc", repeat)
    if key not in _CACHE:
        _CACHE[key] = _build_nc(repeat)
    return _CACHE[key]


def _prep(adj, subevent, event, attn_w):
    adj = np.ascontiguousarray(adj, dtype=np.float32)
    subevent = np.ascontiguousarray(subevent, dtype=np.float32)
    event = np.ascontiguousarray(event, dtype=np.float32)
    attn_w = np.asarray(attn_w, dtype=np.float32)

    a_s, a_e = attn_w[:D], attn_w[D:]
    s_proj = (subevent @ a_s).astype(np.float32)        # [S]
    e_proj = (event @ a_e).astype(np.float32)           # [E]

    # sub16[p, n*D+d] = subevent[n*128+p, d]
    sub16 = (
        subevent.astype(_DT_NP)
        .reshape(SC_TOT, 128, D).transpose(1, 0, 2).reshape(128, SC_TOT * D)
    )
    sub16 = np.ascontiguousarray(sub16)
    # spj[p, n] = s_proj[n*128+p]
    spj = np.ascontiguousarray(s_proj.reshape(SC_TOT, 128).T)

    in_maps = []
    for c in range(N_CORES):
        sl = slice(c * EV, (c + 1) * EV)
        ebt = np.ascontiguousarray(
            np.broadcast_to(e_proj[sl].astype(_DT_NP)[None, :], (128, EV)))
        evh = np.ascontiguousarray(
            (0.5 * event[sl])
            .astype(np.float32)
            .reshape(EVB, 128, D).transpose(1, 0, 2).reshape(128, EVB * D))
        in_maps.append({
            "adj": adj[sl],
            "subt": sub16,
            "spj": spj,
            "ebt": ebt,
            "evh": evh,
        })
    return in_maps


def _make_in_maps(inputs):
    return _prep(inputs["adj"], inputs["subevent"], inputs["event"],
                 inputs["attn_w"])


def kernel(adj, subevent, event, attn_w):
    from concourse.bass_utils import run_bass_kernel_spmd

    in_maps = _prep(adj, subevent, event, attn_w)
    nc = _get_nc()
    res = run_bass_kernel_spmd(nc, in_maps, list(range(N_CORES)))

    out = np.empty((E_TOT, D), dtype=np.float32)
    for c in range(N_CORES):
        o = res.results[c]["out"]  # [128, EVB*D]
        out[c * EV:(c + 1) * EV] = (
            o.reshape(128, EVB, D).transpose(1, 0, 2).reshape(EV, D)
        )
    return out


if __name__ == "__main__":
    rng = np.random.default_rng(0)
    adj = rng.random((E_TOT, S_TOT), dtype=np.float32)
    sub = rng.standard_normal((S_TOT, D), dtype=np.float32)
    ev = rng.standard_normal((E_TOT, D), dtype=np.float32)
    w = rng.uniform(-0.1, 0.1, 2 * D).astype(np.float32)
    out = kernel(adj, sub, ev, w)
    print(out.shape, out.dtype)
